# revision 1
# baseline (speedup 1.0000x reference)
"""CrossTransformer Trainium2 kernel — 8 NeuronCores.

Sharding: core c = (batch b = c//2, head-pair hg = c%2).  Attention is
head-parallel (2 heads/core, fp32r matmuls, exp on ACT with fused
row-sum accum); out-proj + FFN are token-parallel (half of the 2048
tokens per core) after an intra-pair AllToAll of the attention output.

Both softmax directions run the same "column-softmax" pipeline with
swapped inputs (m0 = m1-path(x1, x0)); each path's softmax denominator
is the opposite path's exp row-sum (E_ba = E_ab^T).
"""
import numpy as np

B, NT, E, H, D = 4, 2048, 256, 4, 64
HPC = 2            # heads per core
TH = NT // 2       # token half
HID = 2 * E        # FFN hidden (512)
KCH = E // 128     # 128-chunks of E (2)
N_CORES = 8
LN_EPS = 1e-5

_cache = {}


def _build():
    import concourse.bass as bass
    import concourse.tile as tile
    from concourse import bacc
    import concourse.mybir as mybir

    dt = mybir.dt
    AF = mybir.ActivationFunctionType
    OP = mybir.AluOpType
    f32, f32r = dt.float32, dt.float32r

    nc = bacc.Bacc("TRN2", target_bir_lowering=False, debug=False,
                   num_devices=N_CORES)

    def din(name, shape):
        return nc.dram_tensor(name, shape, f32, kind="ExternalInput").ap()

    x0t = din("x0t", [E, NT])          # x0[b].T
    x1t = din("x1t", [E, NT])
    xf_r = [din(f"x{d}t_ffn", [E, TH]) for d in (0, 1)]   # my token half
    wqk = din("wqk", [E, 128])         # pre-scaled, this core's heads
    bqk = din("bqk", [128, 1])
    wv = din("wv", [E, 256])
    bv = din("bv", [128, 1])
    wo = din("wo", [E, E])
    bo = din("bo", [E, 1])
    w1 = din("w1", [HID, HID])
    w1bar = din("w1bar", [HID, 1])
    b1 = din("b1", [HID, 1])
    b1bar = din("b1bar", [1, 1])
    lng = din("lng", [HID, 1])
    lnb = din("lnb", [HID, 1])
    w2 = din("w2", [HID, E])
    b2 = din("b2", [E, 1])
    ident = din("ident", [128, 128])   # identity matrix
    ones = din("ones", [128, 1])

    outs = [nc.dram_tensor(f"out{d}t", [E, TH], f32, kind="ExternalOutput").ap()
            for d in (0, 1)]

    with tile.TileContext(nc) as tc:
        _body(nc, tc, bass, mybir, tile,
              dict(x0t=x0t, x1t=x1t, xf_r=xf_r, wqk=wqk, bqk=bqk, wv=wv,
                   bv=bv, wo=wo, bo=bo, w1=w1, w1bar=w1bar, b1=b1,
                   b1bar=b1bar, lng=lng, lnb=lnb, w2=w2, b2=b2,
                   ident=ident, ones=ones, outs=outs))
    nc.compile()
    return nc


def _body(nc, tc, bass, mybir, tile, t):
    from contextlib import ExitStack
    dt = mybir.dt
    AF = mybir.ActivationFunctionType
    OP = mybir.AluOpType
    f32, f32r = dt.float32, dt.float32r

    es = ExitStack()
    with es:
        wpool = es.enter_context(tc.tile_pool(name="weights", bufs=1))
        dram = es.enter_context(tc.tile_pool(name="dram", bufs=1, space="DRAM"))

        # ---- weight loads (SWDGE casts fp32 -> fp32r where PE consumes) ----
        def load_r(ap_src, p, fshape, tag):
            til = wpool.tile([p, fshape], f32r, tag=tag, name=tag)
            nc.gpsimd.dma_start(til[:], ap_src)
            return til

        def load_f(ap_src, p, fshape, tag):
            til = wpool.tile([p, fshape], f32, tag=tag, name=tag)
            nc.sync.dma_start(til[:], ap_src)
            return til

        wqk_t = [load_r(t["wqk"][k * 128:(k + 1) * 128, :], 128, 128, f"wqk{k}") for k in range(KCH)]
        wv_t = [load_r(t["wv"][k * 128:(k + 1) * 128, :], 128, 256, f"wv{k}") for k in range(KCH)]
        wo_t = [[load_r(t["wo"][k * 128:(k + 1) * 128, m * 128:(m + 1) * 128], 128, 128, f"wo{k}{m}")
                 for m in range(2)] for k in range(KCH)]
        w1_t = [[load_r(t["w1"][k * 128:(k + 1) * 128, m * 128:(m + 1) * 128], 128, 128, f"w1{k}{m}")
                 for m in range(4)] for k in range(4)]
        w2_t = [[load_r(t["w2"][k * 128:(k + 1) * 128, m * 128:(m + 1) * 128], 128, 128, f"w2{k}{m}")
                 for m in range(2)] for k in range(4)]
        w1bar_t = [load_r(t["w1bar"][k * 128:(k + 1) * 128, :], 128, 1, f"w1b{k}") for k in range(4)]
        ones_t = load_r(t["ones"], 128, 1, "ones")
        ident_t = load_r(t["ident"], 128, 128, "ident")
        bqk_t = load_f(t["bqk"], 128, 1, "bqk")
        bv_t = load_f(t["bv"], 128, 1, "bv")
        bo_t = [load_f(t["bo"][m * 128:(m + 1) * 128, :], 128, 1, f"bo{m}") for m in range(2)]
        b1_t = [load_f(t["b1"][m * 128:(m + 1) * 128, :], 128, 1, f"b1_{m}") for m in range(4)]
        b1bar_t = load_f(t["b1bar"], 1, 1, "b1bar")
        lng_t = [load_f(t["lng"][m * 128:(m + 1) * 128, :], 128, 1, f"lng{m}") for m in range(4)]
        lnb_t = [load_f(t["lnb"][m * 128:(m + 1) * 128, :], 128, 1, f"lnb{m}") for m in range(4)]
        b2_t = [load_f(t["b2"][m * 128:(m + 1) * 128, :], 128, 1, f"b2_{m}") for m in range(2)]

        xffn_r = [[None, None], [None, None]]
        xffn_f = [[None, None], [None, None]]
        for d in range(2):
            for k in range(KCH):
                sl = t["xf_r"][d][k * 128:(k + 1) * 128, :]
                xffn_r[d][k] = load_r(sl, 128, TH, f"xfr{d}{k}")
                xffn_f[d][k] = load_f(sl, 128, TH, f"xff{d}{k}")

        # ================= projections =================
        qkT = [None, None]   # [128(2h*64d), NT] fp32r
        v_t = [[None] * 16, [None] * 16]   # 16 x [128 tok, 128(2h*64d)]
        rows = es.enter_context(tc.tile_pool(name="rows", bufs=1))
        attn_es = ExitStack()
        qp = attn_es.enter_context(tc.tile_pool(name="qkv", bufs=1))
        with tc.tile_pool(name="xfull", bufs=1) as xp, \
             tc.tile_pool(name="qkps", bufs=1, space="PSUM") as qkps, \
             tc.tile_pool(name="vps", bufs=3, space="PSUM") as vps:
            xt = [[None, None], [None, None]]
            for s, src in enumerate((t["x0t"], t["x1t"])):
                for k in range(KCH):
                    xt[s][k] = xp.tile([128, NT], f32r, tag=f"x{s}{k}", name=f"x{s}{k}")
                    nc.gpsimd.dma_start(xt[s][k][:], src[k * 128:(k + 1) * 128, :])
            for s in range(2):
                ps = qkps.tile([128, NT], f32)
                for jn in range(NT // 512):
                    for k in range(KCH):
                        nc.tensor.matmul(ps[:, jn * 512:(jn + 1) * 512],
                                         wqk_t[k][:], xt[s][k][:, jn * 512:(jn + 1) * 512],
                                         start=(k == 0), stop=(k == KCH - 1))
                qkT[s] = qp.tile([128, NT], f32r, tag=f"qkT{s}", name=f"qkT{s}")
                nc.scalar.activation(qkT[s][:], ps[:], AF.Identity, bias=bqk_t[:], scale=1.0)
            for s in range(2):
                for it in range(16):
                    pv = vps.tile([128, 256], f32, tag="vps")
                    for var in range(2):
                        for k in range(KCH):
                            nc.tensor.matmul(pv[:, var * 128:(var + 1) * 128],
                                             xt[s][k][:, it * 128:(it + 1) * 128],
                                             wv_t[k][:, var * 128:(var + 1) * 128],
                                             start=(k == 0), stop=(k == KCH - 1))
                    v_t[s][it] = qp.tile([128, 256], f32r, tag=f"v{s}_{it}", name=f"v{s}_{it}")
                    nc.vector.tensor_copy(v_t[s][it][:], pv[:])

        # ================= attention (two symmetric paths) =================
        # path p: (A,B) = (p, 1-p); output = m for dir (1-p) tokens of x_{1-p}
        mn_pool = attn_es.enter_context(tc.tile_pool(name="mnorm", bufs=1))
        rsj = [rows.tile([128, 64], f32, tag=f"rsj{jj}", name=f"rsj{jj}") for jj in range(2)]
        # row-vector tiles: engine ops need base partition 0 (and equal
        # bases across SBUF operands), so each row vector gets its own tile
        m_raw = [None, None]
        mrp = attn_es.enter_context(tc.tile_pool(name="mraw", bufs=1))
        with tc.tile_pool(name="estrip", bufs=4) as ep, \
             tc.tile_pool(name="simps", bufs=3, space="PSUM") as simps, \
             tc.tile_pool(name="avps", bufs=2, space="PSUM") as avps:
            for p in range(2):
                A, Bi = p, 1 - p
                m_raw[p] = mrp.tile([128, NT], f32, tag=f"mraw{p}", name=f"mraw{p}")
                for jj in range(2):
                    av = [avps.tile([128, 512], f32, tag="av", name=f"av{p}_{jj}_{_i}") for _i in range(2)]
                    for it in range(16):
                        est = [None, None]
                        for h in range(2):
                            sp = simps.tile([128, 1024], f32, tag="sim")
                            for jc in range(2):
                                nc.tensor.matmul(
                                    sp[:, jc * 512:(jc + 1) * 512],
                                    qkT[A][64 * h:64 * (h + 1), it * 128:(it + 1) * 128],
                                    qkT[Bi][64 * h:64 * (h + 1),
                                            jj * 1024 + jc * 512:jj * 1024 + (jc + 1) * 512],
                                    start=True, stop=True,
                                    tile_position=(64 * h, 0))
                            est[h] = ep.tile([128, 1024], f32r, tag="est", name=f"est{h}")
                            col = (p * 2 + h) * 16 + it
                            nc.scalar.activation(est[h][:], sp[:], AF.Exp,
                                                 accum_out=rsj[jj][:, col:col + 1])
                        for jc in range(2):
                            for h in range(2):
                                # lhsT = zero-padded v variant h: rows 64h:64h+64
                                # of the product get head h's AV, rest zeros
                                nc.tensor.matmul(
                                    av[jc][:],
                                    v_t[A][it][:, h * 128:(h + 1) * 128],
                                    est[h][:, jc * 512:(jc + 1) * 512],
                                    start=(it == 0 and h == 0),
                                    stop=(it == 15 and h == 1))
                    for jc in range(2):
                        nc.vector.tensor_copy(
                            m_raw[p][:, jj * 1024 + jc * 512:jj * 1024 + (jc + 1) * 512],
                            av[jc][:])

        # ---- denominators: den[path p] = rowsums of path (1-p) ----
        rsall = rows.tile([128, 64], f32, tag="rsall")
        nc.vector.tensor_add(rsall[:], rsj[0][:], rsj[1][:])
        rsall_r = rows.tile([128, 64], f32r, tag="rsallr")
        nc.vector.tensor_copy(rsall_r[:], rsall[:])
        with tc.tile_pool(name="trps", bufs=1, space="PSUM") as trps:
            tp = trps.tile([64, 128], f32)
            nc.tensor.matmul(tp[:], rsall_r[:], ident_t[:], start=True, stop=True)
            rsT = rows.tile([64, 128], f32, tag="rsT")
            nc.vector.tensor_copy(rsT[:], tp[:])
        den_dram = dram.tile([4, 2048], f32)
        for r in range(4):
            nc.sync.dma_start(
                den_dram[r].rearrange("(it p) -> it p", it=16),
                rsT[r * 16:(r + 1) * 16, :])
        den_rows = mn_pool.tile([4, 2048], f32, tag="denrows", name="denrows")
        nc.sync.dma_start(den_rows[:], den_dram[:])
        lnden = mn_pool.tile([4, 2048], f32, tag="lnden", name="lnden")
        nc.scalar.activation(lnden[:], den_rows[:], AF.Ln)
        recipden = mn_pool.tile([4, 2048], f32, tag="recipden", name="recipden")
        nc.scalar.activation(recipden[:], lnden[:], AF.Exp, scale=-1.0)
        recip_dram = dram.tile([4, 2048], f32)
        nc.sync.dma_start(recip_dram[:], recipden[:])

        # ---- normalize + bv;  den for path p = rows (1-p)*2+h ----
        # DVE cannot take 0-step partition APs, so materialize the row
        # broadcast with a DMA from DRAM (partitions 64h:64h+64 <- head h row).
        m_norm = [None, None]
        for p in range(2):
            recipb = mn_pool.tile([128, NT], f32, tag=f"recipb{p}", name=f"recipb{p}")
            for h in range(2):
                r = (1 - p) * 2 + h
                nc.sync.dma_start(recipb[64 * h:64 * (h + 1), :],
                                  recip_dram[r:r + 1, :].to_broadcast((64, NT)))
            m_norm[p] = mn_pool.tile([128, NT], f32, tag=f"mnorm{p}", name=f"mnorm{p}")
            nc.vector.tensor_mul(m_norm[p][:], m_raw[p][:], recipb[:])
            nc.vector.tensor_scalar(m_norm[p][:], m_norm[p][:], bv_t[:], None, OP.add)

        # ======== exchange: 8-way AllToAll, reshard (b,hg) -> token-eighth ====
        # block r (of 8) = token columns [r*256:(r+1)*256]; after the
        # exchange, core c holds m for ALL batches at ITS 256-token slice.
        # bounce layout: [8 blocks, 2 paths, 128, 256]
        bounce_in = dram.tile([4, 2, 2, 128, 256], f32)   # (b_blk, kc_blk, path, p, t)
        bounce_out = dram.tile([4, 2, 2, 128, 256], f32)
        for p in range(2):
            for bb in range(4):
                for kb in range(2):
                    nc.sync.dma_start(
                        bounce_in[bb, kb, p],
                        m_norm[p][:, (2 * bb + kb) * 256:(2 * bb + kb + 1) * 256])
        nc.gpsimd.collective_compute(
            "AllToAll", mybir.AluOpType.bypass,
            replica_groups=[list(range(8))],
            ins=[bounce_in.opt()], outs=[bounce_out.opt()])
        attn_es.close()   # frees qkT/v/m_raw/m_norm SBUF for the FFN phase
        # out block s = from core s=(b=s//2, hg=s%2): m[batch b, heads hg, my toks]
        # m_dir[d][kc][:, b*256:(b+1)*256] = bounce_out[2b+kc, 1-d]
        mdir = [[None, None], [None, None]]   # [dir][kc] -> [128, TH=4x256] f32r
        mpool = es.enter_context(tc.tile_pool(name="mdir", bufs=1))
        for d in range(2):
            p = 1 - d
            for kc in range(2):
                mdir[d][kc] = mpool.tile([128, TH], f32r, tag=f"mdir{d}{kc}", name=f"mdir{d}{kc}")
                for bb in range(4):
                    nc.gpsimd.dma_start(mdir[d][kc][:, bb * 256:(bb + 1) * 256],
                                        bounce_out[bb, kc, p])

        # ================= out-projection =================
        mproj = [[None, None], [None, None]]
        with tc.tile_pool(name="mpps", bufs=2, space="PSUM") as mpps:
            for d in range(2):
                for mo in range(2):
                    ps = mpps.tile([128, TH], f32, tag="mp")
                    for nn in range(2):
                        for kc in range(2):
                            nc.tensor.matmul(ps[:, nn * 512:(nn + 1) * 512],
                                             wo_t[kc][mo][:],
                                             mdir[d][kc][:, nn * 512:(nn + 1) * 512],
                                             start=(kc == 0), stop=(kc == 1))
                    mproj[d][mo] = mpool.tile([128, TH], f32r, tag=f"mproj{d}{mo}", name=f"mproj{d}{mo}")
                    nc.scalar.activation(mproj[d][mo][:], ps[:], AF.Identity,
                                         bias=bo_t[mo][:], scale=1.0)

        # ================= FFN =================
        # ccT chunks (f32r): [xffn_r[d][0], xffn_r[d][1], mproj[d][0], mproj[d][1]]
        hsb_pool = es.enter_context(tc.tile_pool(name="hsb", bufs=1))
        hsb = {}
        statp = es.enter_context(tc.tile_pool(name="statrows", bufs=1))
        mu_all = statp.tile([1, 2048], f32, tag="muall", name="muall")
        ss_all = statp.tile([1, 2048], f32, tag="srowA", name="ssall")
        with tc.tile_pool(name="sq", bufs=3) as sqp, \
             tc.tile_pool(name="hps", bufs=4, space="PSUM") as hps, \
             tc.tile_pool(name="rowps", bufs=2, space="PSUM") as rowps:
            for d in range(2):
                cc = [xffn_r[d][0], xffn_r[d][1], mproj[d][0], mproj[d][1]]
                for tcn in range(2):
                    sl = slice(tcn * 512, (tcn + 1) * 512)
                    col = (d * 2 + tcn) * 512
                    pmu = rowps.tile([1, 512], f32, tag="pmu")
                    for kc in range(4):
                        nc.tensor.matmul(pmu[:], w1bar_t[kc][:], cc[kc][:, sl],
                                         start=(kc == 0), stop=(kc == 3))
                    nc.vector.tensor_scalar(mu_all[0:1, col:col + 512], pmu[:],
                                            b1bar_t[:], None, OP.add)
                    pss = rowps.tile([1, 512], f32, tag="pss")
                    for mh in range(4):
                        ph = hps.tile([128, 512], f32, tag="ph")
                        for kc in range(4):
                            nc.tensor.matmul(ph[:], w1_t[kc][mh][:], cc[kc][:, sl],
                                             start=(kc == 0), stop=(kc == 3))
                        hkey = (d, tcn, mh)
                        hsb[hkey] = hsb_pool.tile([128, 512], f32, tag=f"h{d}{tcn}{mh}", name=f"h{d}{tcn}{mh}")
                        nc.vector.tensor_scalar(hsb[hkey][:], ph[:], b1_t[mh][:],
                                                None, OP.add)
                        sq = sqp.tile([128, 512], f32r, tag="sq")
                        nc.vector.tensor_mul(sq[:], hsb[hkey][:], hsb[hkey][:])
                        nc.tensor.matmul(pss[:], ones_t[:], sq[:],
                                         start=(mh == 0), stop=(mh == 3))
                    nc.vector.tensor_copy(ss_all[0:1, col:col + 512], pss[:])

        # batched LN stats: rstd = exp(-0.5 ln(ss/512 - mu^2 + eps))
        musq = statp.tile([1, 2048], f32, tag="srowB", name="musq")
        nc.vector.tensor_mul(musq[:], mu_all[:], mu_all[:])
        ve = statp.tile([1, 2048], f32, tag="srowC", name="ve")
        nc.vector.scalar_tensor_tensor(ve[:], ss_all[:], 1.0 / HID, musq[:],
                                       OP.mult, OP.subtract)
        vee = statp.tile([1, 2048], f32, tag="srowA", name="vee")
        nc.vector.tensor_scalar(vee[:], ve[:], LN_EPS, None, OP.add)
        lnve = statp.tile([1, 2048], f32, tag="srowB", name="lnve")
        nc.scalar.activation(lnve[:], vee[:], AF.Ln)
        rstd = statp.tile([1, 2048], f32, tag="srowA", name="rstd")
        nc.scalar.activation(rstd[:], lnve[:], AF.Exp, scale=-0.5)
        murstd = statp.tile([1, 2048], f32, tag="srowB", name="murstd")
        nc.vector.tensor_mul(murstd[:], mu_all[:], rstd[:])
        # materialize partition-broadcasts of rstd/murstd via DRAM
        stat_dram = dram.tile([2, 2048], f32)
        nc.sync.dma_start(stat_dram[0:1, :], rstd[:])
        nc.sync.dma_start(stat_dram[1:2, :], murstd[:])
        statb = es.enter_context(tc.tile_pool(name="statb", bufs=1))
        rstdb = statb.tile([128, 2048], f32, tag="rstdb", name="rstdb")
        murstdb = statb.tile([128, 2048], f32, tag="murstdb", name="murstdb")
        nc.sync.dma_start(rstdb[:], stat_dram[0:1, :].to_broadcast((128, 2048)))
        nc.sync.dma_start(murstdb[:], stat_dram[1:2, :].to_broadcast((128, 2048)))

        # affine + gelu + W2 + residual
        with tc.tile_pool(name="uacts", bufs=3) as up, \
             tc.tile_pool(name="gacts", bufs=5) as gp, \
             tc.tile_pool(name="osb", bufs=4) as op_, \
             tc.tile_pool(name="ops", bufs=2, space="PSUM") as ops:
            for d in range(2):
                for tcn in range(2):
                    sl = slice(tcn * 512, (tcn + 1) * 512)
                    col = (d * 2 + tcn) * 512
                    rsl = rstdb[:, col:col + 512]
                    msl = murstdb[:, col:col + 512]
                    gh = [None] * 4
                    for mh in range(4):
                        u = up.tile([128, 512], f32, tag="u")
                        nc.vector.tensor_mul(u[:], hsb[(d, tcn, mh)][:], rsl[:])
                        t2 = up.tile([128, 512], f32, tag="t2")
                        nc.vector.tensor_sub(t2[:], u[:], msl[:])
                        gh[mh] = gp.tile([128, 512], f32r, tag="gh", name=f"gh{mh}")
                        nc.scalar.activation(gh[mh][:], t2[:], AF.Gelu,
                                             bias=lnb_t[mh][:], scale=lng_t[mh][:])
                    for mo in range(2):
                        po = ops.tile([128, 512], f32, tag="po")
                        for kh in range(4):
                            nc.tensor.matmul(po[:], w2_t[kh][mo][:], gh[kh][:],
                                             start=(kh == 0), stop=(kh == 3))
                        ot = op_.tile([128, 512], f32, tag="ot")
                        nc.vector.scalar_tensor_tensor(
                            ot[:], po[:], b2_t[mo][:], xffn_f[d][mo][:, sl],
                            OP.add, OP.add)
                        nc.sync.dma_start(t["outs"][d][mo * 128:(mo + 1) * 128, sl], ot[:])


def _host_prep(inputs):
    """Build per-core in_maps from full inputs."""
    x0 = np.asarray(inputs["x0"], np.float32)
    x1 = np.asarray(inputs["x1"], np.float32)
    Wqk = np.asarray(inputs["Wqk"], np.float32) * (D ** -0.25)
    bqk = np.asarray(inputs["bqk"], np.float32) * (D ** -0.25)
    Wv = np.asarray(inputs["Wv"], np.float32)
    bv = np.asarray(inputs["bv"], np.float32)
    Wo = np.asarray(inputs["Wo"], np.float32)
    bo = np.asarray(inputs["bo"], np.float32)
    W1 = np.asarray(inputs["W1"], np.float32)
    b1 = np.asarray(inputs["b1"], np.float32)
    lng = np.asarray(inputs["ln_g"], np.float32)
    lnb = np.asarray(inputs["ln_b"], np.float32)
    W2 = np.asarray(inputs["W2"], np.float32)
    b2 = np.asarray(inputs["b2"], np.float32)

    shared = {
        "wo": np.ascontiguousarray(Wo),
        "bo": bo.reshape(E, 1),
        "w1": np.ascontiguousarray(W1),
        "w1bar": W1.mean(axis=1).reshape(HID, 1),
        "b1": b1.reshape(HID, 1),
        "b1bar": np.array([[b1.mean()]], np.float32),
        "lng": lng.reshape(HID, 1),
        "lnb": lnb.reshape(HID, 1),
        "w2": np.ascontiguousarray(W2),
        "b2": b2.reshape(E, 1),
        "ident": np.eye(128, dtype=np.float32),
        "ones": np.ones((128, 1), np.float32),
    }
    in_maps = []
    for c in range(N_CORES):
        b, hg = c // 2, c % 2
        hs = slice(hg * 128, hg * 128 + 128)
        ts = slice(hg * TH, hg * TH + TH)
        m = dict(shared)
        m["x0t"] = np.ascontiguousarray(x0[b].T)
        m["x1t"] = np.ascontiguousarray(x1[b].T)
        # FFN slice: my 256-token slice of EVERY batch, columns (b, t) b-major
        cs = slice(c * 256, (c + 1) * 256)
        m["x0t_ffn"] = np.ascontiguousarray(
            x0[:, cs, :].reshape(B * 256, E).T)
        m["x1t_ffn"] = np.ascontiguousarray(
            x1[:, cs, :].reshape(B * 256, E).T)
        m["wqk"] = np.ascontiguousarray(Wqk[:, hs])
        m["bqk"] = bqk[hs].reshape(128, 1)
        wvp = np.zeros((E, 256), np.float32)
        wvp[:, 0:64] = Wv[:, hg * 128:hg * 128 + 64]        # head0 -> cols 0:64
        wvp[:, 192:256] = Wv[:, hg * 128 + 64:hg * 128 + 128]  # head1 -> cols 192:256
        m["wv"] = wvp
        m["bv"] = bv[hs].reshape(128, 1)
        in_maps.append(m)
    return in_maps


def _get_nc():
    if "nc" not in _cache:
        _cache["nc"] = _build()
    return _cache["nc"]


def kernel(**inputs):
    from concourse import bass_utils
    nc = _get_nc()
    in_maps = _host_prep(inputs)
    res = bass_utils.run_bass_kernel_spmd(nc, in_maps, core_ids=list(range(N_CORES)))
    out0 = np.empty((B, NT, E), np.float32)
    out1 = np.empty((B, NT, E), np.float32)
    for c in range(N_CORES):
        cs = slice(c * 256, (c + 1) * 256)
        o0 = res.results[c]["out0t"]  # [E, 4*256], cols (b, t)
        o1 = res.results[c]["out1t"]
        for b in range(B):
            out0[b, cs, :] = o0[:, b * 256:(b + 1) * 256].T
            out1[b, cs, :] = o1[:, b * 256:(b + 1) * 256].T
    return out0, out1



# revision 3
# speedup vs baseline: 1.9024x; 1.9024x over previous
"""CrossTransformer Trainium2 kernel — 8 NeuronCores, v2.

Sharding: core c = (batch b = c//2, head-pair hg = c%2).  Attention is
head-parallel (2 heads/core); out-proj + FFN are token-parallel: core c
owns tokens {c*128..c*128+128} u {1024+c*128..+128} of every batch.

Both softmax directions run the same column-softmax pipeline with
swapped inputs; each path's softmax denominator is the opposite path's
exp row-sum.  Path1's m (dir-0 output) is normalized with path0's
row-sums (available early), exchanged in two half-AllToAlls that
overlap path1's attention tail; path0's m (dir-1) goes last.  bv is
folded into the out-projection bias on the host (bo2 = bo + Wo^T bv).
"""
import numpy as np

B, NT, E, H, D = 4, 2048, 256, 4, 64
TH = NT // 2       # FFN tokens per core (4 batches x 256)
HID = 2 * E
KCH = E // 128     # 2
N_CORES = 8
LN_EPS = 1e-5

_cache = {}


def _build():
    import concourse.bass as bass
    import concourse.tile as tile
    from concourse import bacc
    import concourse.mybir as mybir

    dt = mybir.dt
    f32, bf16 = dt.float32, dt.bfloat16

    nc = bacc.Bacc("TRN2", target_bir_lowering=False, debug=False,
                   num_devices=N_CORES)

    def din(name, shape, dtype=bf16):
        return nc.dram_tensor(name, shape, dtype, kind="ExternalInput").ap()

    t = dict(
        x0t=din("x0t", [E, NT]),
        x1t=din("x1t", [E, NT]),
        xf=[din(f"x{d}t_ffn", [E, TH]) for d in (0, 1)],
        wqk=din("wqk", [E, 128]),
        bqk=din("bqk", [128, 1], f32),
        wv=din("wv", [E, 256]),
        wo=din("wo", [E, E]),
        bo2=din("bo2", [E, 1], f32),
        w1=din("w1", [HID, HID]),
        w1bar=din("w1bar", [HID, 1]),
        b1=din("b1", [HID, 1], f32),
        b1bar=din("b1bar", [1, 1], f32),
        lng=din("lng", [HID, 1], f32),
        lnb=din("lnb", [HID, 1], f32),
        w2=din("w2", [HID, E]),
        b2=din("b2", [E, 1], f32),
        ident=din("ident", [128, 128]),
        ones=din("ones", [128, 1]),
        outs=[nc.dram_tensor(f"out{d}t", [E, TH], bf16,
                             kind="ExternalOutput").ap() for d in (0, 1)],
    )

    with tile.TileContext(nc) as tc:
        _body(nc, tc, bass, mybir, tile, t)
    nc.compile()
    return nc


def _body(nc, tc, bass, mybir, tile, t):
    from contextlib import ExitStack
    dt = mybir.dt
    AF = mybir.ActivationFunctionType
    OP = mybir.AluOpType
    f32, bf16 = dt.float32, dt.bfloat16

    es = ExitStack()
    with es:
        wpool = es.enter_context(tc.tile_pool(name="weights", bufs=1))
        dram = es.enter_context(tc.tile_pool(name="dram", bufs=1, space="DRAM"))

        qs = [nc.sync, nc.gpsimd]

        def load(ap_src, p, fshape, tag, dtype=bf16, q=0):
            til = wpool.tile([p, fshape], dtype, tag=tag, name=tag)
            qs[q].dma_start(til[:], ap_src)
            return til

        # ---- group 1: attention weights + x (alternate the two DGE queues)
        wqk_t = [load(t["wqk"][k * 128:(k + 1) * 128, :], 128, 128, f"wqk{k}", q=k % 2)
                 for k in range(KCH)]
        bqk_t = load(t["bqk"], 128, 1, "bqk", f32, 0)
        xt = [[None, None], [None, None]]
        for s, src in enumerate((t["x0t"], t["x1t"])):
            for k in range(KCH):
                xt[s][k] = load(src[k * 128:(k + 1) * 128, :], 128, NT,
                                f"x{s}{k}", q=(s + k) % 2)
        wv_t = [load(t["wv"][k * 128:(k + 1) * 128, :], 128, 256, f"wv{k}", q=(k + 1) % 2)
                for k in range(KCH)]
        ident_t = load(t["ident"], 128, 128, "ident", bf16, 1)

        # ---- group 2: out-proj + FFN weights (stream during attention)
        wo_t = [[load(t["wo"][k * 128:(k + 1) * 128, m * 128:(m + 1) * 128],
                      128, 128, f"wo{k}{m}", q=(k + m) % 2)
                 for m in range(2)] for k in range(KCH)]
        w1_t = [[load(t["w1"][k * 128:(k + 1) * 128, m * 128:(m + 1) * 128],
                      128, 128, f"w1{k}{m}", q=(k + m) % 2)
                 for m in range(4)] for k in range(4)]
        w2_t = [[load(t["w2"][k * 128:(k + 1) * 128, m * 128:(m + 1) * 128],
                      128, 128, f"w2{k}{m}", q=(k + m) % 2)
                 for m in range(2)] for k in range(4)]
        w1bar_t = [load(t["w1bar"][k * 128:(k + 1) * 128, :], 128, 1, f"w1b{k}", q=k % 2)
                   for k in range(4)]
        ones_t = load(t["ones"], 128, 1, "ones", bf16, 0)
        bo2_t = [load(t["bo2"][m * 128:(m + 1) * 128, :], 128, 1, f"bo2{m}", f32, m % 2)
                 for m in range(2)]
        b1_t = [load(t["b1"][m * 128:(m + 1) * 128, :], 128, 1, f"b1_{m}", f32, m % 2)
                for m in range(4)]
        b1bar_t = load(t["b1bar"], 1, 1, "b1bar", f32, 0)
        lng_t = [load(t["lng"][m * 128:(m + 1) * 128, :], 128, 1, f"lng{m}", f32, m % 2)
                 for m in range(4)]
        lnb_t = [load(t["lnb"][m * 128:(m + 1) * 128, :], 128, 1, f"lnb{m}", f32, (m + 1) % 2)
                 for m in range(4)]
        b2_t = [load(t["b2"][m * 128:(m + 1) * 128, :], 128, 1, f"b2_{m}", f32, m % 2)
                for m in range(2)]
        xffn = [[load(t["xf"][d][k * 128:(k + 1) * 128, :], 128, TH, f"xf{d}{k}",
                      q=(d + k) % 2) for k in range(KCH)] for d in range(2)]

        # ================= projections =================
        rows = es.enter_context(tc.tile_pool(name="rows", bufs=1))
        attn_es = ExitStack()
        qp = attn_es.enter_context(tc.tile_pool(name="qkv", bufs=1))
        qkT = [None, None]
        v_t = [[None] * 16, [None] * 16]
        with tc.tile_pool(name="qkps", bufs=1, space="PSUM") as qkps, \
             tc.tile_pool(name="vps", bufs=2, space="PSUM") as vps:
            for s in range(2):
                ps = qkps.tile([128, NT], f32)
                for jn in range(4):
                    for k in range(KCH):
                        nc.tensor.matmul(ps[:, jn * 512:(jn + 1) * 512],
                                         wqk_t[k][:], xt[s][k][:, jn * 512:(jn + 1) * 512],
                                         start=(k == 0), stop=(k == KCH - 1))
                qkT[s] = qp.tile([128, NT], bf16, tag=f"qkT{s}", name=f"qkT{s}")
                nc.scalar.activation(qkT[s][:], ps[:], AF.Identity, bias=bqk_t[:], scale=1.0)
            for s in range(2):
                for it in range(16):
                    pv = vps.tile([128, 256], f32, tag="vps")
                    for k in range(KCH):
                        nc.tensor.matmul(pv[:], xt[s][k][:, it * 128:(it + 1) * 128],
                                         wv_t[k][:], start=(k == 0), stop=(k == KCH - 1))
                    v_t[s][it] = qp.tile([128, 256], bf16, tag=f"v{s}_{it}", name=f"v{s}_{it}")
                    nc.vector.tensor_copy(v_t[s][it][:], pv[:])

        # ================= attention =================
        # path p: est_p[j_p-part, j_(1-p)-free]; m_raw[p] = dir-(1-p) output.
        # den for m_raw[p] = exp row-sums accumulated during path (1-p).
        mn_pool = attn_es.enter_context(tc.tile_pool(name="mnorm", bufs=1))
        rsj = [rows.tile([128, 64], f32, tag=f"rsj{jj}", name=f"rsj{jj}") for jj in range(2)]
        m_raw0 = mn_pool.tile([128, NT], bf16, tag="mraw0", name="mraw0")  # dir1, raw
        m1n = mn_pool.tile([128, NT], bf16, tag="m1n", name="m1n")         # dir0, normalized
        m0n = mn_pool.tile([128, NT], bf16, tag="m0n", name="m0n")         # dir1, normalized
        recipb = [None, None]   # [p] -> [128, NT] bf16 recip rows for m_raw[p]
        recip_dram = dram.tile([4, 2048], bf16, name="recip_dram")   # row = pden*2 + h

        # bounce buffers: dir0 in two half exchanges, dir1 in one
        bnc_in = [dram.tile([8, 128, 128], bf16, name=f"bncin{_i}") for _i in range(2)]
        bnc_out = [dram.tile([8, 128, 128], bf16, name=f"bncout{_i}") for _i in range(2)]
        bnc3_in = dram.tile([8, 2, 128, 128], bf16, name="bnc3_in")
        bnc3_out = dram.tile([8, 2, 128, 128], bf16, name="bnc3_out")

        def make_recipb(pden, trp):
            """recip of rsall cols for path pden -> broadcast [128, NT] tile.
            pden = path whose row-sums these are = normalizes m_raw[1-pden]."""
            c0 = pden * 32
            rsall = rows.tile([128, 32], f32, tag=f"rsall{pden}", name=f"rsall{pden}")
            nc.vector.tensor_add(rsall[:], rsj[0][:, c0:c0 + 32], rsj[1][:, c0:c0 + 32])
            rec = rows.tile([128, 32], f32, tag=f"rec{pden}", name=f"rec{pden}")
            nc.vector.reciprocal(rec[:], rsall[:])
            recb = rows.tile([128, 32], bf16, tag=f"recb{pden}", name=f"recb{pden}")
            nc.vector.tensor_copy(recb[:], rec[:])
            tp = trp.tile([32, 128], f32, tag="trp", name=f"trp{pden}")
            nc.tensor.matmul(tp[:], recb[:], ident_t[:], start=True, stop=True)
            rsT = rows.tile([32, 128], bf16, tag=f"rsT{pden}", name=f"rsT{pden}")
            nc.vector.tensor_copy(rsT[:], tp[:])
            for h in range(2):
                nc.sync.dma_start(
                    recip_dram[pden * 2 + h].rearrange("(it p) -> it p", it=16),
                    rsT[h * 16:(h + 1) * 16, :])
            rb = mn_pool.tile([128, NT], bf16, tag=f"recipb{pden}", name=f"recipb{pden}")
            for h in range(2):
                nc.sync.dma_start(rb[64 * h:64 * (h + 1), :],
                                  recip_dram[pden * 2 + h:pden * 2 + h + 1, :]
                                  .to_broadcast((64, NT)))
            return rb

        with tc.tile_pool(name="estrip", bufs=3) as ep, \
             tc.tile_pool(name="simps", bufs=2, space="PSUM") as simps, \
             tc.tile_pool(name="avps", bufs=2, space="PSUM") as avps, \
             tc.tile_pool(name="trps", bufs=1, space="PSUM") as trps:
            for p in range(2):
                A, Bi = p, 1 - p
                for jj in range(2):
                    av = [avps.tile([128, 512], f32, tag="av", name=f"av{p}_{jj}_{i}")
                          for i in range(2)]
                    for it in range(16):
                        est = ep.tile([128, 2048], bf16, tag="est", name="est")
                        for h in range(2):
                            sp = simps.tile([128, 1024], f32, tag="sim")
                            for jc in range(2):
                                nc.tensor.matmul(
                                    sp[:, jc * 512:(jc + 1) * 512],
                                    qkT[A][64 * h:64 * (h + 1), it * 128:(it + 1) * 128],
                                    qkT[Bi][64 * h:64 * (h + 1),
                                            jj * 1024 + jc * 512:jj * 1024 + (jc + 1) * 512],
                                    start=True, stop=True,
                                    tile_position=(64 * h, 0))
                            nc.scalar.activation(est[:, h * 1024:(h + 1) * 1024], sp[:], AF.Exp)
                        col = p * 32 + it
                        # rows sums for h0,h1 -> rsj cols (p*2+h)*16+it (stride 16)
                        nc.vector.tensor_reduce(
                            rsj[jj][:, col:col + 17:16],
                            est[:].rearrange("p (h u) -> p h u", h=2),
                            mybir.AxisListType.X, OP.add)
                        for h in range(2):
                            for jc in range(2):
                                nc.tensor.matmul(
                                    av[jc][:],
                                    v_t[A][it][:, h * 128:(h + 1) * 128],
                                    est[:, h * 1024 + jc * 512:h * 1024 + (jc + 1) * 512],
                                    start=(it == 0 and h == 0),
                                    stop=(it == 15 and h == 1))
                    if p == 0:
                        # raw copy; normalization must wait for path1 row-sums
                        for jc in range(2):
                            nc.vector.tensor_copy(
                                m_raw0[:, jj * 1024 + jc * 512:jj * 1024 + (jc + 1) * 512],
                                av[jc][:])
                    else:
                        # den (= path0 row-sums) is ready: normalize + bounce now
                        for jc in range(2):
                            sl = slice(jj * 1024 + jc * 512, jj * 1024 + (jc + 1) * 512)
                            nc.vector.tensor_mul(m1n[:, sl], av[jc][:], recipb[1][:, sl])
                        nc.sync.dma_start(
                            bnc_in[jj].rearrange("r p u -> p r u"),
                            m1n[:, jj * 1024:(jj + 1) * 1024]
                            .rearrange("p (r u) -> p r u", r=8))
                        nc.gpsimd.collective_compute(
                            "AllToAll", mybir.AluOpType.bypass,
                            replica_groups=[list(range(8))],
                            ins=[bnc_in[jj].opt()], outs=[bnc_out[jj].opt()])
                if p == 0:
                    # path0 done -> den for m_raw[1] (made before path1 uses it)
                    recipb[1] = make_recipb(0, trps)
            # path1 done -> den for m_raw[0]; normalize + exchange dir1
            recipb[0] = make_recipb(1, trps)
            nc.vector.tensor_mul(m0n[:], m_raw0[:], recipb[0][:])
            for hf in range(2):
                nc.sync.dma_start(
                    bnc3_in.rearrange("r h p u -> h p r u")[hf],
                    m0n[:, hf * 1024:(hf + 1) * 1024]
                    .rearrange("p (r u) -> p r u", r=8))
            nc.gpsimd.collective_compute(
                "AllToAll", mybir.AluOpType.bypass,
                replica_groups=[list(range(8))],
                ins=[bnc3_in.opt()], outs=[bnc3_out.opt()])

        # ---- unpack: mdir[d][kc] [128, TH]; col = b*256 + half*128 + u ----
        attn_es.close()
        mpool = es.enter_context(tc.tile_pool(name="mdir", bufs=1))
        mdir = [[None, None], [None, None]]
        for d in range(2):
            for kc in range(2):
                mdir[d][kc] = mpool.tile([128, TH], bf16, tag=f"mdir{d}{kc}",
                                         name=f"mdir{d}{kc}")
        # mdir cols are (hf, b, u): col = hf*512 + b*128 + u
        for hf in range(2):
            for kc in range(2):
                nc.gpsimd.dma_start(
                    mdir[0][kc][:, hf * 512:(hf + 1) * 512]
                    .rearrange("p (b u) -> p b u", b=4),
                    bnc_out[hf].rearrange("(b k) p u -> k p b u", k=2)[kc])
        for hf in range(2):
            for kc in range(2):
                nc.gpsimd.dma_start(
                    mdir[1][kc][:, hf * 512:(hf + 1) * 512]
                    .rearrange("p (b u) -> p b u", b=4),
                    bnc3_out.rearrange("(b k) h p u -> k h p b u", k=2)[kc, hf])

        # ================= out-projection + FFN (pipelined per direction) ====
        hsb_pool = es.enter_context(tc.tile_pool(name="hsb", bufs=2))
        statp = es.enter_context(tc.tile_pool(name="statrows", bufs=4))
        stat_dram = dram.tile([4, 2, 512], bf16, name="stat_dram")
        mproj = [[None, None], [None, None]]
        with tc.tile_pool(name="mpps", bufs=2, space="PSUM") as mpps, \
             tc.tile_pool(name="sq", bufs=3) as sqp, \
             tc.tile_pool(name="hps", bufs=2, space="PSUM") as hps, \
             tc.tile_pool(name="rowps", bufs=1, space="PSUM") as rowps, \
             tc.tile_pool(name="uacts", bufs=2) as up, \
             tc.tile_pool(name="gacts", bufs=5) as gp, \
             tc.tile_pool(name="osb", bufs=4) as op_, \
             tc.tile_pool(name="ops", bufs=2, space="PSUM") as ops, \
             tc.tile_pool(name="statb", bufs=2) as sbp:
            for d in range(2):
                for mo in range(2):
                    mproj[d][mo] = mpool.tile([128, TH], bf16, tag=f"mproj{d}{mo}",
                                              name=f"mproj{d}{mo}")
                for nn2 in range(2):
                    for mo in range(2):
                        psm = mpps.tile([128, 512], f32, tag="mp")
                        for kc in range(2):
                            nc.tensor.matmul(psm[:],
                                             wo_t[kc][mo][:],
                                             mdir[d][kc][:, nn2 * 512:(nn2 + 1) * 512],
                                             start=(kc == 0), stop=(kc == 1))
                        nc.scalar.activation(mproj[d][mo][:, nn2 * 512:(nn2 + 1) * 512],
                                             psm[:], AF.Identity,
                                             bias=bo2_t[mo][:], scale=1.0)
                cc = [xffn[d][0], xffn[d][1], mproj[d][0], mproj[d][1]]
                for tcn in range(2):
                    un = d * 2 + tcn
                    sl = slice(tcn * 512, (tcn + 1) * 512)
                    pmu = rowps.tile([1, 512], f32, tag="pmu")
                    for kc in range(4):
                        nc.tensor.matmul(pmu[:], w1bar_t[kc][:], cc[kc][:, sl],
                                         start=(kc == 0), stop=(kc == 3))
                    mu = statp.tile([1, 512], f32, tag="mu", name=f"mu{un}")
                    nc.vector.tensor_scalar(mu[:], pmu[:], b1bar_t[:], None, OP.add)
                    pss = rowps.tile([1, 512], f32, tag="pss")
                    hsb = [None] * 4
                    for mh in range(4):
                        ph = hps.tile([128, 512], f32, tag="ph")
                        for kc in range(4):
                            nc.tensor.matmul(ph[:], w1_t[kc][mh][:], cc[kc][:, sl],
                                             start=(kc == 0), stop=(kc == 3))
                        hsb[mh] = hsb_pool.tile([128, 512], f32, tag=f"h{mh}",
                                                name=f"h{un}_{mh}")
                        nc.vector.tensor_scalar(hsb[mh][:], ph[:], b1_t[mh][:],
                                                None, OP.add)
                        sq = sqp.tile([128, 512], bf16, tag="sq")
                        nc.scalar.activation(sq[:], hsb[mh][:], AF.Square)
                        nc.tensor.matmul(pss[:], ones_t[:], sq[:],
                                         start=(mh == 0), stop=(mh == 3))
                    ss = statp.tile([1, 512], f32, tag="ss", name=f"ss{un}")
                    nc.vector.tensor_copy(ss[:], pss[:])
                    musq = statp.tile([1, 512], f32, tag="musq", name=f"musq{un}")
                    nc.vector.tensor_mul(musq[:], mu[:], mu[:])
                    ve = statp.tile([1, 512], f32, tag="ve", name=f"ve{un}")
                    nc.vector.scalar_tensor_tensor(ve[:], ss[:], 1.0 / HID, musq[:],
                                                   OP.mult, OP.subtract)
                    vee = statp.tile([1, 512], f32, tag="vee", name=f"vee{un}")
                    nc.vector.tensor_scalar(vee[:], ve[:], LN_EPS, None, OP.add)
                    lnve = statp.tile([1, 512], f32, tag="lnve", name=f"lnve{un}")
                    nc.scalar.activation(lnve[:], vee[:], AF.Ln)
                    rstd = statp.tile([1, 512], f32, tag="rstd", name=f"rstd{un}")
                    nc.scalar.activation(rstd[:], lnve[:], AF.Exp, scale=-0.5)
                    rstd_b = statp.tile([1, 512], bf16, tag="rstdb16", name=f"rstdb16{un}")
                    nc.vector.tensor_copy(rstd_b[:], rstd[:])
                    murstd = statp.tile([1, 512], bf16, tag="murstd", name=f"murstd{un}")
                    nc.vector.tensor_mul(murstd[:], mu[:], rstd[:])
                    nc.sync.dma_start(stat_dram[un, 0:1], rstd_b[:])
                    nc.sync.dma_start(stat_dram[un, 1:2], murstd[:])
                    rstdb = sbp.tile([128, 512], bf16, tag="rstdbb", name=f"rstdbb{un}")
                    murstdb = sbp.tile([128, 512], bf16, tag="murstdbb", name=f"murstdbb{un}")
                    nc.gpsimd.dma_start(rstdb[:], stat_dram[un, 0:1].to_broadcast((128, 512)))
                    nc.gpsimd.dma_start(murstdb[:], stat_dram[un, 1:2].to_broadcast((128, 512)))
                    gh = [None] * 4
                    for mh in range(4):
                        eng = nc.vector if mh % 2 == 0 else nc.gpsimd
                        u = up.tile([128, 512], f32, tag="u")
                        eng.tensor_mul(u[:], hsb[mh][:], rstdb[:])
                        t2 = up.tile([128, 512], f32, tag="t2")
                        eng.tensor_sub(t2[:], u[:], murstdb[:])
                        gh[mh] = gp.tile([128, 512], bf16, tag="gh", name=f"gh{mh}")
                        nc.scalar.activation(gh[mh][:], t2[:], AF.Gelu,
                                             bias=lnb_t[mh][:], scale=lng_t[mh][:])
                    for mo in range(2):
                        po = ops.tile([128, 512], f32, tag="po")
                        for kh in range(4):
                            nc.tensor.matmul(po[:], w2_t[kh][mo][:], gh[kh][:],
                                             start=(kh == 0), stop=(kh == 3))
                        ot = op_.tile([128, 512], bf16, tag="ot")
                        nc.vector.scalar_tensor_tensor(
                            ot[:], po[:], b2_t[mo][:], xffn[d][mo][:, sl],
                            OP.add, OP.add)
                        nc.sync.dma_start(t["outs"][d][mo * 128:(mo + 1) * 128, sl], ot[:])


def _host_prep(inputs):
    from ml_dtypes import bfloat16 as bf
    x0 = np.asarray(inputs["x0"], np.float32)
    x1 = np.asarray(inputs["x1"], np.float32)
    Wqk = np.asarray(inputs["Wqk"], np.float32) * (D ** -0.25)
    bqk = np.asarray(inputs["bqk"], np.float32) * (D ** -0.25)
    Wv = np.asarray(inputs["Wv"], np.float32)
    bv = np.asarray(inputs["bv"], np.float32)
    Wo = np.asarray(inputs["Wo"], np.float32)
    bo = np.asarray(inputs["bo"], np.float32)
    W1 = np.asarray(inputs["W1"], np.float32)
    b1 = np.asarray(inputs["b1"], np.float32)
    lng = np.asarray(inputs["ln_g"], np.float32)
    lnb = np.asarray(inputs["ln_b"], np.float32)
    W2 = np.asarray(inputs["W2"], np.float32)
    b2 = np.asarray(inputs["b2"], np.float32)
    bo2 = bo + bv @ Wo

    shared = {
        "wo": Wo.astype(bf),
        "bo2": bo2.reshape(E, 1).astype(np.float32),
        "w1": W1.astype(bf),
        "w1bar": W1.mean(axis=1).reshape(HID, 1).astype(bf),
        "b1": b1.reshape(HID, 1),
        "b1bar": np.array([[b1.mean()]], np.float32),
        "lng": lng.reshape(HID, 1),
        "lnb": lnb.reshape(HID, 1),
        "w2": W2.astype(bf),
        "b2": b2.reshape(E, 1),
        "ident": np.eye(128).astype(bf),
        "ones": np.ones((128, 1), np.float32).astype(bf),
    }
    in_maps = []
    for c in range(N_CORES):
        b, hg = c // 2, c % 2
        hs = slice(hg * 128, hg * 128 + 128)
        m = dict(shared)
        m["x0t"] = np.ascontiguousarray(x0[b].T).astype(bf)
        m["x1t"] = np.ascontiguousarray(x1[b].T).astype(bf)
        # FFN slice: tokens {hf*1024 + c*128 + u}; cols (hf, b, u)
        def _xf(x):
            xs = np.stack([x[:, c * 128:(c + 1) * 128, :],
                           x[:, 1024 + c * 128:1024 + (c + 1) * 128, :]])  # [2,B,128,E]
            return np.ascontiguousarray(xs.transpose(3, 0, 1, 2).reshape(E, TH)).astype(bf)
        m["x0t_ffn"] = _xf(x0)
        m["x1t_ffn"] = _xf(x1)
        m["wqk"] = np.ascontiguousarray(Wqk[:, hs]).astype(bf)
        m["bqk"] = bqk[hs].reshape(128, 1)
        wvp = np.zeros((E, 256), np.float32)
        wvp[:, 0:64] = Wv[:, hg * 128:hg * 128 + 64]
        wvp[:, 192:256] = Wv[:, hg * 128 + 64:hg * 128 + 128]
        m["wv"] = wvp.astype(bf)
        in_maps.append(m)
    return in_maps


def _get_nc():
    if "nc" not in _cache:
        _cache["nc"] = _build()
    return _cache["nc"]


def _unshard(getter):
    out0 = np.empty((B, NT, E), np.float32)
    out1 = np.empty((B, NT, E), np.float32)
    for c in range(N_CORES):
        o0 = np.asarray(getter(c, 0), np.float32)  # [E, TH] cols (b, half, u)
        o1 = np.asarray(getter(c, 1), np.float32)
        for b in range(B):
            for hf in range(2):
                ts = slice(hf * 1024 + c * 128, hf * 1024 + (c + 1) * 128)
                cs = slice(hf * 512 + b * 128, hf * 512 + (b + 1) * 128)
                out0[b, ts, :] = o0[:, cs].T
                out1[b, ts, :] = o1[:, cs].T
    return out0, out1


def kernel(**inputs):
    from concourse import bass_utils
    nc = _get_nc()
    in_maps = _host_prep(inputs)
    res = bass_utils.run_bass_kernel_spmd(nc, in_maps, core_ids=list(range(N_CORES)))
    return _unshard(lambda c, d: res.results[c][f"out{d}t"])
